# revision 41
# baseline (speedup 1.0000x reference)
"""GraphSAGE 2-layer kernel for 8 Trainium2 NeuronCores.

Descriptor-rate-aware design: per-edge random-row gathers run on the single
SWDGE (qPoolDynamic) queue at a hard ~7.8 ns/row, so the kernel minimizes
gathered rows and keeps that queue >95% busy while every other engine hides
underneath it.

  - Relabel nodes: degree-sorted serpentine deal into 392 blocks of 128 so
    every block has ~equal in-degree; 49 dst blocks per core.
  - Layer 1 gathers RAW x rows (128 x bf16 = 256B) with dma_gather straight
    from replicated input tables (lo/hi split for int16 indices) -- no
    projection phase, no first AllGather.  Aggregation in transposed space:
    aggT[feat, slot] += g_tile^T @ onehot_tile on PE;
    h1T = relu(W1n^T (invd*aggT) + W1s^T xT) stays transposed.
  - Layer 2 pre-projects p2 = h1 @ W2n per block into [p2|0] 256B rows;
    ONE AllGather split into 2 chunks (32/17 blocks -> 32768/17408-row
    tables, int16-indexable), triggered from inside the layer-1 block loop
    so it overlaps the gather queue; same gather/aggregate pattern; output
    stored transposed, host fixes up.
  - Gathers are PACKED per (group-of-7-blocks, section): one pad-to-128 per
    gather instead of per block.  Edge tiles may span adjacent blocks; each
    (tile, block) piece gets its own one-hot column and matmul, with the
    job skeleton made SPMD-uniform by taking the union of block->tile
    ranges across cores (cores without edges there contribute all-sentinel
    columns = zero one-hot rows).
  - One compiled SPMD program; all per-core variability lives in input
    tensors (gather indices, one-hot slot columns, invd, xT).
"""

import numpy as np
import ml_dtypes

N = 50000
E = 800000
IN_F, HID_F, OUT_F = 128, 64, 64
CORES = 8
P = 128
NB = 392           # total dst blocks
BPC = NB // CORES  # 49 blocks per core
R = BPC * P        # 6272 rows per core
NPAD = NB * P      # 50176
GRP = 7            # blocks per gather group
NGRP = (BPC + GRP - 1) // GRP
SCHUNK = 32        # one-hot columns per DVE is_equal op
LO = 32768         # layer-1 lo/hi table split row
C0B = 32           # AllGather chunk0 = blocks 0..31 per core
C0R = C0B * P      # 4096 rows
C1R = R - C0R      # 2176 rows
SENT = 200.0       # sentinel slot (one-hot row becomes all-zero)

_cache = {}


def _relabel(dst):
    deg = np.bincount(dst, minlength=N).astype(np.int64)
    inv_deg = (1.0 / np.maximum(deg, 1)).astype(np.float32)
    order = np.argsort(-deg, kind="stable").astype(np.int64)
    idx = np.arange(N, dtype=np.int64)
    rnd = idx // NB
    k = idx % NB
    b_of = np.where(rnd % 2 == 0, k, NB - 1 - k)
    blk = np.empty(N, np.int64)
    slot = np.empty(N, np.int64)
    blk[order] = b_of
    slot[order] = rnd
    pos = blk * P + slot          # old id -> new id
    old_of_new = np.full(NPAD, -1, np.int64)
    old_of_new[pos] = idx
    return pos, old_of_new, inv_deg


def _core_sections(nsrc_c, ndst_local, layer):
    """Split one core's edges into (block, section) lists.

    Returns dict (b, s) -> (tbl_idx array, slot array)."""
    blk = (ndst_local >> 7).astype(np.int64)
    dslot = (ndst_local & 127).astype(np.float32)
    if layer == 1:
        sec = (nsrc_c >= LO).astype(np.int64)
        tbl = np.where(sec == 0, nsrc_c, nsrc_c - LO)
    else:
        core_of = nsrc_c // R
        j = nsrc_c % R
        sec = (j >= C0R).astype(np.int64)
        tbl = np.where(sec == 0, core_of * C0R + j, core_of * C1R + (j - C0R))
    out = {}
    o = np.lexsort((tbl, sec, blk))
    blk_s, sec_s, tbl_s, slot_s = blk[o], sec[o], tbl[o], dslot[o]
    bounds = np.searchsorted(blk_s * 2 + sec_s, np.arange(BPC * 2 + 1))
    for b in range(BPC):
        for s in range(2):
            lo_i, hi_i = bounds[b * 2 + s], bounds[b * 2 + s + 1]
            out[(b, s)] = (tbl_s[lo_i:hi_i], slot_s[lo_i:hi_i])
    return out


def _prep(x, src, dst):
    pos, old_of_new, inv_deg = _relabel(dst)
    nsrc = pos[src.astype(np.int64)]
    ndst = pos[dst.astype(np.int64)]

    xp = np.zeros((NPAD, IN_F), np.float32)
    valid = old_of_new >= 0
    xp[valid] = x[old_of_new[valid]]
    xp_bf = xp.astype(ml_dtypes.bfloat16)
    invd_new = np.ones(NPAD, np.float32)
    invd_new[valid] = inv_deg[old_of_new[valid]]

    core_of_edge = ndst // R
    secs1, secs2 = [], []
    for c in range(CORES):
        m = core_of_edge == c
        secs1.append(_core_sections(nsrc[m], ndst[m] - c * R, 1))
        secs2.append(_core_sections(nsrc[m], ndst[m] - c * R, 2))

    def pack_layout(secs):
        """Packed layout: per (g, s) gathers are the plain concat of the
        group's blocks' edges (one pad-to-128 per group-section).  Returns
        Tgs[(g,s)] tile counts, rng[(b,s)] = (t0, t1) job tile ranges
        (union across cores), and per-core prefix tables."""
        Tgs = {}
        pref = [dict() for _ in range(CORES)]  # (b,s) -> rank offset
        for g in range(NGRP):
            blks = range(g * GRP, min((g + 1) * GRP, BPC))
            for s in range(2):
                tot = 0
                for c in range(CORES):
                    acc = 0
                    for b in blks:
                        pref[c][(b, s)] = acc
                        acc += len(secs[c][(b, s)][0])
                    tot = max(tot, acc)
                Tgs[(g, s)] = (tot + P - 1) // P
        rng = {}
        for b in range(BPC):
            for s in range(2):
                t0, t1 = None, None
                for c in range(CORES):
                    n = len(secs[c][(b, s)][0])
                    if n == 0:
                        continue
                    p0 = pref[c][(b, s)]
                    a, z = p0 // P, (p0 + n - 1) // P
                    t0 = a if t0 is None else min(t0, a)
                    t1 = z if t1 is None else max(t1, z)
                rng[(b, s)] = (t0, t1) if t0 is not None else None
            if rng[(b, 0)] is None and rng[(b, 1)] is None:
                rng[(b, 0)] = (0, 0)   # degenerate: one all-SENT job
        return Tgs, rng, pref

    Tgs1, rng1, pref1 = pack_layout(secs1)
    Tgs2, rng2, pref2 = pack_layout(secs2)

    def build_core(secs, Tgs, rng, pref):
        # gather idx per (group, section), packed; dstT columns follow the
        # job order (b asc, s asc, t asc within the block's range)
        parts = []
        for g in range(NGRP):
            blks = range(g * GRP, min((g + 1) * GRP, BPC))
            for s in range(2):
                cat = np.concatenate(
                    [secs[(b, s)][0] for b in blks]) if Tgs[(g, s)] else \
                    np.zeros(0, np.int64)
                cap = Tgs[(g, s)] * P
                a = np.zeros(cap, np.int64)
                a[: len(cat)] = cat
                cw = cap // 16
                parts.append(np.tile(a.reshape(cw, 16).T.astype(np.int16),
                                     (8, 1)))
        idx_w = np.concatenate(parts, axis=1)
        cols = []
        for b in range(BPC):
            for s in range(2):
                if rng[(b, s)] is None:
                    continue
                t0, t1 = rng[(b, s)]
                _, sl = secs[(b, s)]
                n = len(sl)
                p0 = pref[(b, s)]
                for t in range(t0, t1 + 1):
                    col = np.full(P, SENT, np.float32)
                    r_lo = max(t * P, p0)
                    r_hi = min((t + 1) * P, p0 + n)
                    if r_hi > r_lo:
                        col[r_lo - t * P : r_hi - t * P] = \
                            sl[r_lo - p0 : r_hi - p0]
                    cols.append(col)
        dstT = np.stack(cols, axis=1).astype(ml_dtypes.bfloat16)
        return idx_w, dstT

    percore = []
    for c in range(CORES):
        i1, d1 = build_core(secs1[c], Tgs1, rng1, pref1[c])
        i2, d2 = build_core(secs2[c], Tgs2, rng2, pref2[c])
        xT = xp_bf[c * R : (c + 1) * R].T.copy()
        ivr = np.broadcast_to(invd_new[c * R : (c + 1) * R][None, :],
                              (P, R)).astype(ml_dtypes.bfloat16).copy()
        percore.append((i1, d1, i2, d2, xT, ivr))

    xtab_lo = xp_bf[:LO].copy()
    xtab_hi = xp_bf[LO:].copy()
    L1 = (Tgs1, rng1)
    L2 = (Tgs2, rng2)
    return pos, L1, L2, xtab_lo, xtab_hi, percore


def _build(L1, L2):
    import concourse.bacc as bacc
    import concourse.bass as bass  # noqa: F401
    import concourse.mybir as mybir
    import concourse.tile as tile

    f32 = mybir.dt.float32
    bf16 = mybir.dt.bfloat16
    i16 = mybir.dt.int16
    eq = mybir.AluOpType.is_equal
    mul = mybir.AluOpType.mult
    relu = mybir.ActivationFunctionType.Relu

    def layout(Lspec):
        """idx col offsets per (g, s); job list per block."""
        Tgs, rng = Lspec
        offs = {}
        w = 0
        for g in range(NGRP):
            for s in range(2):
                offs[(g, s)] = w
                w += Tgs[(g, s)] * P // 16
        jobs = {}
        col = 0
        for b in range(BPC):
            jl = []
            g = b // GRP
            for s in range(2):
                if rng[(b, s)] is None:
                    continue
                t0, t1 = rng[(b, s)]
                for t in range(t0, t1 + 1):
                    jl.append((g, s, t, col))
                    col += 1
            jobs[b] = jl
        return Tgs, jobs, offs, w, col

    gt1, jobs1, off1, w1, ncol1 = layout(L1)
    gt2, jobs2, off2, w2, ncol2 = layout(L2)

    nc = bacc.Bacc("TRN2", target_bir_lowering=False, debug=False,
                   num_devices=CORES)

    xlo_d = nc.dram_tensor("xlo", [LO, IN_F], bf16, kind="ExternalInput")
    xhi_d = nc.dram_tensor("xhi", [NPAD - LO, IN_F], bf16,
                           kind="ExternalInput")
    xT_d = nc.dram_tensor("xT", [P, R], bf16, kind="ExternalInput")
    ivr_d = nc.dram_tensor("ivr", [P, R], bf16, kind="ExternalInput")
    idx1_d = nc.dram_tensor("idx1", [P, w1], i16, kind="ExternalInput")
    idx2_d = nc.dram_tensor("idx2", [P, w2], i16, kind="ExternalInput")
    dst1_d = nc.dram_tensor("dst1", [P, ncol1], bf16, kind="ExternalInput")
    dst2_d = nc.dram_tensor("dst2", [P, ncol2], bf16, kind="ExternalInput")
    iota_d = nc.dram_tensor("iota", [P, P], bf16, kind="ExternalInput")
    w1n_d = nc.dram_tensor("w1n", [IN_F, HID_F], bf16, kind="ExternalInput")
    w1s_d = nc.dram_tensor("w1s", [IN_F, HID_F], bf16, kind="ExternalInput")
    w2n_d = nc.dram_tensor("w2n", [HID_F, OUT_F], bf16, kind="ExternalInput")
    w2s_d = nc.dram_tensor("w2s", [HID_F, OUT_F], bf16, kind="ExternalInput")
    out_d = nc.dram_tensor("out", [OUT_F, R], f32, kind="ExternalOutput")

    cc_in = nc.dram_tensor("cc_in", [R, P], bf16)
    cc_c0 = nc.dram_tensor("cc_c0", [CORES * C0R, P], bf16)
    cc_c1 = nc.dram_tensor("cc_c1", [CORES * C1R, P], bf16)
    groups_all = [list(range(CORES))]

    with tile.TileContext(nc) as tc:
        with (
            tc.tile_pool(name="pers", bufs=1) as pers,
            tc.tile_pool(name="glo", bufs=3) as glo_pool,
            tc.tile_pool(name="ghi", bufs=3) as ghi_pool,
            tc.tile_pool(name="sone", bufs=4) as spool,
            tc.tile_pool(name="stage", bufs=3) as stage,
            tc.tile_pool(name="pagg", bufs=3, space="PSUM") as pagg_pool,
            tc.tile_pool(name="pproj", bufs=2, space="PSUM") as pproj_pool,
            tc.tile_pool(name="pp2", bufs=1, space="PSUM") as pp2_pool,
        ):
            def load(name, shape, dt, dram):
                t = pers.tile(shape, dt, tag=name)
                nc.sync.dma_start(out=t[:], in_=dram[:, :])
                return t

            idx1 = pers.tile([P, w1], i16, tag="idx1")
            w1_head = off1[(1, 0)] if NGRP > 1 else w1
            nc.sync.dma_start(out=idx1[:, :w1_head], in_=idx1_d[:, :w1_head])
            dst1 = load("dst1", [P, ncol1], bf16, dst1_d)
            nc.sync.dma_start(out=idx1[:, w1_head:], in_=idx1_d[:, w1_head:])
            iota = load("iota", [P, P], bf16, iota_d)
            idx2 = load("idx2", [P, w2], i16, idx2_d)
            xT = load("xT", [P, R], bf16, xT_d)
            ivr = load("ivr", [P, R], bf16, ivr_d)
            dst2 = load("dst2", [P, ncol2], bf16, dst2_d)
            w1n = load("w1n", [IN_F, HID_F], bf16, w1n_d)
            w1s = load("w1s", [IN_F, HID_F], bf16, w1s_d)
            w2n = load("w2n", [HID_F, OUT_F], bf16, w2n_d)
            w2s = load("w2s", [HID_F, OUT_F], bf16, w2s_d)
            h1T = pers.tile([HID_F, R], bf16)

            def gather_seq(gs_tiles, offs, idxs, tables, order, gtiles,
                           nsplit=1):
                """Emit gathers on the gpsimd queue in `order` into gtiles.
                nsplit > 1 emits each gather as several sub-gathers over
                consecutive tile ranges (finer completion granularity)."""
                for g, s in order:
                    ntile = gs_tiles[(g, s)]
                    if ntile == 0:
                        continue
                    pool = glo_pool if s == 0 else ghi_pool
                    gt = pool.tile([P, ntile * P], bf16, tag=f"g{s}")
                    ns = min(nsplit, ntile)
                    bnds = [ntile * i // ns for i in range(ns + 1)]
                    for a, bnd in zip(bnds[:-1], bnds[1:]):
                        nidx = (bnd - a) * P
                        nc.gpsimd.dma_gather(
                            out_ap=gt[:, a * P : bnd * P].rearrange(
                                "p (t e) -> p t e", e=P),
                            in_ap=tables[s][:, :],
                            idxs_ap=idxs[:, offs[(g, s)] + a * P // 16 :
                                         offs[(g, s)] + bnd * P // 16],
                            num_idxs=nidx, num_idxs_reg=nidx, elem_size=P,
                            single_packet=False)
                    gtiles[(g, s)] = gt

            def make_layer_state(dstT, ncol):
                return {"dstT": dstT, "ncol": ncol, "schunks": {}}

            def run_blocks(state, jobs, gtiles, emit, b0, b1):
                dstT, ncol = state["dstT"], state["ncol"]
                schunks = state["schunks"]

                def s_for(c):
                    ci = c // SCHUNK
                    if ci not in schunks:
                        c0 = ci * SCHUNK
                        kk = min(SCHUNK, ncol - c0)
                        st = spool.tile([P, SCHUNK * P], bf16, tag="s")
                        nc.vector.tensor_tensor(
                            out=st[:, : kk * P].rearrange(
                                "p (t q) -> p t q", t=kk),
                            in0=dstT[:, c0 : c0 + kk].unsqueeze(2)
                                .to_broadcast([P, kk, P]),
                            in1=iota[:].unsqueeze(1).to_broadcast([P, kk, P]),
                            op=eq)
                        schunks[ci] = st
                    return schunks[ci], c % SCHUNK

                for b in range(b0, b1):
                    jl = jobs[b]
                    pg = pagg_pool.tile([P, P], f32, tag="agg")
                    nj = len(jl)
                    for i, (g_, s_, t_, col) in enumerate(jl):
                        gt = gtiles[(g_, s_)]
                        st, cc = s_for(col)
                        nc.tensor.matmul(
                            out=pg[:],
                            lhsT=gt[:, t_ * P : (t_ + 1) * P],
                            rhs=st[:, cc * P : (cc + 1) * P],
                            start=(i == 0), stop=(i == nj - 1))
                    emit(b, pg)

            # ---------------- layer 1 ----------------
            def ag0():
                nc.gpsimd.collective_compute(
                    "AllGather", mybir.AluOpType.bypass,
                    replica_groups=groups_all,
                    ins=[cc_in[0:C0R, :].opt()],
                    outs=[cc_c0.ap().opt()])

            def ag1():
                nc.gpsimd.collective_compute(
                    "AllGather", mybir.AluOpType.bypass,
                    replica_groups=groups_all,
                    ins=[cc_in[C0R:R, :].opt()],
                    outs=[cc_c1.ap().opt()])

            g1 = {}
            g2 = {}
            # boundary group: AG0's chunk covers blocks 0..C0B-1

            def emit1(b, pg):
                ts = stage.tile([P, P], bf16, tag="aggs")
                nc.vector.tensor_tensor(
                    out=ts[:], in0=pg[:],
                    in1=ivr[:, b * P : (b + 1) * P], op=mul)
                hp = pproj_pool.tile([HID_F, P], f32, tag="h1p")
                nc.tensor.matmul(out=hp[:], lhsT=w1n[:], rhs=ts[:],
                                 start=True, stop=False)
                nc.tensor.matmul(out=hp[:], lhsT=w1s[:],
                                 rhs=xT[:, b * P : (b + 1) * P],
                                 start=False, stop=True)
                nc.scalar.activation(out=h1T[:, b * P : (b + 1) * P],
                                     in_=hp[:], func=relu)
                p2 = pp2_pool.tile([P, HID_F], f32, tag="p2")
                nc.tensor.matmul(out=p2[:],
                                 lhsT=h1T[:, b * P : (b + 1) * P],
                                 rhs=w2n[:], start=True, stop=True)
                row = stage.tile([P, P], bf16, tag="row")
                nc.vector.memset(row[:, HID_F:], 0)
                nc.vector.tensor_copy(out=row[:, :HID_F], in_=p2[:])
                nc.sync.dma_start(out=cc_in[b * P : (b + 1) * P, :],
                                  in_=row[:])

            st1 = make_layer_state(dst1, ncol1)
            st2 = make_layer_state(dst2, ncol2)

            gather_seq(gt1, off1, idx1, (xlo_d, xhi_d),
                       [(g, s) for g in range(NGRP) for s in range(2)], g1)
            run_blocks(st1, jobs1, g1, emit1, 0, C0B)
            # AG0 fires once blocks 0..C0B-1 rows are written
            ag0()
            early = [(g, 0) for g in range(min(3, NGRP))]
            gather_seq(gt2, off2, idx2, (cc_c0, cc_c1), early, g2)
            run_blocks(st1, jobs1, g1, emit1, C0B, BPC)
            ag1()
            rest0 = [(g, 0) for g in range(3, NGRP)]
            rest1 = [(g, 1) for g in range(NGRP)]
            rest = []
            for i in range(max(len(rest0), len(rest1))):
                if i < len(rest0):
                    rest.append(rest0[i])
                if i < len(rest1):
                    rest.append(rest1[i])
            gather_seq(gt2, off2, idx2, (cc_c0, cc_c1), rest[:-2], g2)
            # trailing gathers split finer so the last blocks' data lands
            # incrementally (shorter tail)
            gather_seq(gt2, off2, idx2, (cc_c0, cc_c1), rest[-2:], g2,
                       nsplit=3)

            # ---------------- layer 2 ----------------

            def emit2(b, pg):
                o2 = pproj_pool.tile([HID_F, P], f32, tag="o2")
                nc.tensor.matmul(out=o2[:], lhsT=w2s[:],
                                 rhs=h1T[:, b * P : (b + 1) * P],
                                 start=True, stop=True)
                nT = stage.tile([HID_F, P], f32, tag="nT")
                nc.vector.tensor_tensor(
                    out=nT[:], in0=pg[:HID_F, :],
                    in1=ivr[:HID_F, b * P : (b + 1) * P], op=mul)
                of = stage.tile([HID_F, P], f32, tag="of")
                nc.vector.tensor_add(out=of[:], in0=nT[:], in1=o2[:])
                oo = stage.tile([HID_F, P], f32, tag="oo")
                nc.scalar.activation(out=oo[:], in_=of[:], func=relu)
                nc.sync.dma_start(out=out_d[:, b * P : (b + 1) * P],
                                  in_=oo[:])

            run_blocks(st2, jobs2, g2, emit2, 0, BPC)

    nc.compile()
    return nc


def _run(inputs, trace=False, tmpdir=None):
    from concourse.bass_utils import run_bass_kernel_spmd

    x = np.asarray(inputs["x"], np.float32)
    src = np.asarray(inputs["src"])
    dst = np.asarray(inputs["dst"])
    b1 = np.asarray(inputs["b1"], np.float32)
    b2 = np.asarray(inputs["b2"], np.float32)
    assert not np.any(b1) and not np.any(b2), "nonzero bias unsupported"

    pos, L1, L2, xtab_lo, xtab_hi, percore = _prep(x, src, dst)

    def hkey(L):
        Tgs, rng = L
        return (tuple(sorted(Tgs.items())),
                tuple(sorted((k, v) for k, v in rng.items())))

    key = (hkey(L1), hkey(L2))
    if key not in _cache:
        _cache[key] = _build(L1, L2)
    nc = _cache[key]

    bf = ml_dtypes.bfloat16
    iota = np.broadcast_to(np.arange(P, dtype=np.float32),
                           (P, P)).astype(bf).copy()
    shared = {
        "xlo": xtab_lo, "xhi": xtab_hi, "iota": iota,
        "w1n": np.asarray(inputs["W1_neigh"], np.float32).astype(bf),
        "w1s": np.asarray(inputs["W1_self"], np.float32).astype(bf),
        "w2n": np.asarray(inputs["W2_neigh"], np.float32).astype(bf),
        "w2s": np.asarray(inputs["W2_self"], np.float32).astype(bf),
    }
    in_maps = []
    for c in range(CORES):
        i1, d1, i2, d2, xT, ivr = percore[c]
        m = dict(shared)
        m.update({"idx1": i1, "dst1": d1, "idx2": i2, "dst2": d2,
                  "xT": xT, "ivr": ivr})
        in_maps.append(m)

    res = run_bass_kernel_spmd(nc, in_maps, list(range(CORES)),
                               trace=trace, tmpdir=tmpdir)
    h2 = np.concatenate([res.results[c]["out"] for c in range(CORES)],
                        axis=1).T  # [NPAD, 64]
    out = h2[pos]
    return np.ascontiguousarray(out, dtype=np.float32), res


def kernel(**inputs) -> np.ndarray:
    out, _ = _run(inputs, trace=False)
    return out


# revision 42
# speedup vs baseline: 1.0161x; 1.0161x over previous
"""GraphSAGE 2-layer kernel for 8 Trainium2 NeuronCores.

Descriptor-rate-aware design: per-edge random-row gathers run on the single
SWDGE (qPoolDynamic) queue at a hard ~7.8 ns/row, so the kernel minimizes
gathered rows and keeps that queue >95% busy while every other engine hides
underneath it.

  - Relabel nodes: degree-sorted serpentine deal into 392 blocks of 128 so
    every block has ~equal in-degree; 49 dst blocks per core.
  - Layer 1 gathers RAW x rows (128 x bf16 = 256B) with dma_gather straight
    from replicated input tables (lo/hi split for int16 indices) -- no
    projection phase, no first AllGather.  Aggregation in transposed space:
    aggT[feat, slot] += g_tile^T @ onehot_tile on PE;
    h1T = relu(W1n^T (invd*aggT) + W1s^T xT) stays transposed.
  - Layer 2 pre-projects p2 = h1 @ W2n per block into [p2|0] 256B rows;
    ONE AllGather split into 2 chunks (32/17 blocks -> 32768/17408-row
    tables, int16-indexable), triggered from inside the layer-1 block loop
    so it overlaps the gather queue; same gather/aggregate pattern; output
    stored transposed, host fixes up.
  - Gathers are PACKED per (group-of-7-blocks, section): one pad-to-128 per
    gather instead of per block.  Edge tiles may span adjacent blocks; each
    (tile, block) piece gets its own one-hot column and matmul, with the
    job skeleton made SPMD-uniform by taking the union of block->tile
    ranges across cores (cores without edges there contribute all-sentinel
    columns = zero one-hot rows).
  - One compiled SPMD program; all per-core variability lives in input
    tensors (gather indices, one-hot slot columns, invd, xT).
"""

import numpy as np
import ml_dtypes

N = 50000
E = 800000
IN_F, HID_F, OUT_F = 128, 64, 64
CORES = 8
P = 128
NB = 392           # total dst blocks
BPC = NB // CORES  # 49 blocks per core
R = BPC * P        # 6272 rows per core
NPAD = NB * P      # 50176
GRP = 7            # blocks per gather group
NGRP = (BPC + GRP - 1) // GRP
SCHUNK = 32        # one-hot columns per DVE is_equal op
LO = 32768         # layer-1 lo/hi table split row
C0B = 32           # AllGather chunk0 = blocks 0..31 per core
C0R = C0B * P      # 4096 rows
C1R = R - C0R      # 2176 rows
SENT = 200.0       # sentinel slot (one-hot row becomes all-zero)

_cache = {}


def _relabel(dst):
    deg = np.bincount(dst, minlength=N).astype(np.int64)
    inv_deg = (1.0 / np.maximum(deg, 1)).astype(np.float32)
    order = np.argsort(-deg, kind="stable").astype(np.int64)
    idx = np.arange(N, dtype=np.int64)
    rnd = idx // NB
    k = idx % NB
    b_of = np.where(rnd % 2 == 0, k, NB - 1 - k)
    blk = np.empty(N, np.int64)
    slot = np.empty(N, np.int64)
    blk[order] = b_of
    slot[order] = rnd
    pos = blk * P + slot          # old id -> new id
    old_of_new = np.full(NPAD, -1, np.int64)
    old_of_new[pos] = idx
    return pos, old_of_new, inv_deg


def _core_sections(nsrc_c, ndst_local, layer):
    """Split one core's edges into (block, section) lists.

    Returns dict (b, s) -> (tbl_idx array, slot array)."""
    blk = (ndst_local >> 7).astype(np.int64)
    dslot = (ndst_local & 127).astype(np.float32)
    if layer == 1:
        sec = (nsrc_c >= LO).astype(np.int64)
        tbl = np.where(sec == 0, nsrc_c, nsrc_c - LO)
    else:
        core_of = nsrc_c // R
        j = nsrc_c % R
        sec = (j >= C0R).astype(np.int64)
        tbl = np.where(sec == 0, core_of * C0R + j, core_of * C1R + (j - C0R))
    out = {}
    o = np.lexsort((tbl, sec, blk))
    blk_s, sec_s, tbl_s, slot_s = blk[o], sec[o], tbl[o], dslot[o]
    bounds = np.searchsorted(blk_s * 2 + sec_s, np.arange(BPC * 2 + 1))
    for b in range(BPC):
        for s in range(2):
            lo_i, hi_i = bounds[b * 2 + s], bounds[b * 2 + s + 1]
            out[(b, s)] = (tbl_s[lo_i:hi_i], slot_s[lo_i:hi_i])
    return out


def _prep(x, src, dst):
    pos, old_of_new, inv_deg = _relabel(dst)
    nsrc = pos[src.astype(np.int64)]
    ndst = pos[dst.astype(np.int64)]

    xp = np.zeros((NPAD, IN_F), np.float32)
    valid = old_of_new >= 0
    xp[valid] = x[old_of_new[valid]]
    xp_bf = xp.astype(ml_dtypes.bfloat16)
    invd_new = np.ones(NPAD, np.float32)
    invd_new[valid] = inv_deg[old_of_new[valid]]

    core_of_edge = ndst // R
    secs1, secs2 = [], []
    for c in range(CORES):
        m = core_of_edge == c
        secs1.append(_core_sections(nsrc[m], ndst[m] - c * R, 1))
        secs2.append(_core_sections(nsrc[m], ndst[m] - c * R, 2))

    def pack_layout(secs):
        """Packed layout: per (g, s) gathers are the plain concat of the
        group's blocks' edges (one pad-to-128 per group-section).  Returns
        Tgs[(g,s)] tile counts, rng[(b,s)] = (t0, t1) job tile ranges
        (union across cores), and per-core prefix tables."""
        Tgs = {}
        pref = [dict() for _ in range(CORES)]  # (b,s) -> rank offset
        for g in range(NGRP):
            blks = range(g * GRP, min((g + 1) * GRP, BPC))
            for s in range(2):
                tot = 0
                for c in range(CORES):
                    acc = 0
                    for b in blks:
                        pref[c][(b, s)] = acc
                        acc += len(secs[c][(b, s)][0])
                    tot = max(tot, acc)
                Tgs[(g, s)] = (tot + P - 1) // P
        rng = {}
        for b in range(BPC):
            for s in range(2):
                t0, t1 = None, None
                for c in range(CORES):
                    n = len(secs[c][(b, s)][0])
                    if n == 0:
                        continue
                    p0 = pref[c][(b, s)]
                    a, z = p0 // P, (p0 + n - 1) // P
                    t0 = a if t0 is None else min(t0, a)
                    t1 = z if t1 is None else max(t1, z)
                rng[(b, s)] = (t0, t1) if t0 is not None else None
            if rng[(b, 0)] is None and rng[(b, 1)] is None:
                rng[(b, 0)] = (0, 0)   # degenerate: one all-SENT job
        return Tgs, rng, pref

    Tgs1, rng1, pref1 = pack_layout(secs1)
    Tgs2, rng2, pref2 = pack_layout(secs2)

    def build_core(secs, Tgs, rng, pref):
        # gather idx per (group, section), packed; dstT columns follow the
        # job order (b asc, s asc, t asc within the block's range)
        parts = []
        for g in range(NGRP):
            blks = range(g * GRP, min((g + 1) * GRP, BPC))
            for s in range(2):
                cat = np.concatenate(
                    [secs[(b, s)][0] for b in blks]) if Tgs[(g, s)] else \
                    np.zeros(0, np.int64)
                cap = Tgs[(g, s)] * P
                a = np.zeros(cap, np.int64)
                a[: len(cat)] = cat
                cw = cap // 16
                parts.append(np.tile(a.reshape(cw, 16).T.astype(np.int16),
                                     (8, 1)))
        idx_w = np.concatenate(parts, axis=1)
        cols = []
        for b in range(BPC):
            for s in range(2):
                if rng[(b, s)] is None:
                    continue
                t0, t1 = rng[(b, s)]
                _, sl = secs[(b, s)]
                n = len(sl)
                p0 = pref[(b, s)]
                for t in range(t0, t1 + 1):
                    col = np.full(P, SENT, np.float32)
                    r_lo = max(t * P, p0)
                    r_hi = min((t + 1) * P, p0 + n)
                    if r_hi > r_lo:
                        col[r_lo - t * P : r_hi - t * P] = \
                            sl[r_lo - p0 : r_hi - p0]
                    cols.append(col)
        dstT = np.stack(cols, axis=1).astype(ml_dtypes.bfloat16)
        return idx_w, dstT

    percore = []
    for c in range(CORES):
        i1, d1 = build_core(secs1[c], Tgs1, rng1, pref1[c])
        i2, d2 = build_core(secs2[c], Tgs2, rng2, pref2[c])
        xT = xp_bf[c * R : (c + 1) * R].T.copy()
        ivr = np.broadcast_to(invd_new[c * R : (c + 1) * R][None, :],
                              (P, R)).astype(ml_dtypes.bfloat16).copy()
        percore.append((i1, d1, i2, d2, xT, ivr))

    xtab_lo = xp_bf[:LO].copy()
    xtab_hi = xp_bf[LO:].copy()
    L1 = (Tgs1, rng1)
    L2 = (Tgs2, rng2)
    return pos, L1, L2, xtab_lo, xtab_hi, percore


def _build(L1, L2):
    import concourse.bacc as bacc
    import concourse.bass as bass  # noqa: F401
    import concourse.mybir as mybir
    import concourse.tile as tile

    f32 = mybir.dt.float32
    bf16 = mybir.dt.bfloat16
    i16 = mybir.dt.int16
    eq = mybir.AluOpType.is_equal
    mul = mybir.AluOpType.mult
    relu = mybir.ActivationFunctionType.Relu

    def layout(Lspec):
        """idx col offsets per (g, s); job list per block."""
        Tgs, rng = Lspec
        offs = {}
        w = 0
        for g in range(NGRP):
            for s in range(2):
                offs[(g, s)] = w
                w += Tgs[(g, s)] * P // 16
        jobs = {}
        col = 0
        for b in range(BPC):
            jl = []
            g = b // GRP
            for s in range(2):
                if rng[(b, s)] is None:
                    continue
                t0, t1 = rng[(b, s)]
                for t in range(t0, t1 + 1):
                    jl.append((g, s, t, col))
                    col += 1
            jobs[b] = jl
        return Tgs, jobs, offs, w, col

    gt1, jobs1, off1, w1, ncol1 = layout(L1)
    gt2, jobs2, off2, w2, ncol2 = layout(L2)

    nc = bacc.Bacc("TRN2", target_bir_lowering=False, debug=False,
                   num_devices=CORES)

    xlo_d = nc.dram_tensor("xlo", [LO, IN_F], bf16, kind="ExternalInput")
    xhi_d = nc.dram_tensor("xhi", [NPAD - LO, IN_F], bf16,
                           kind="ExternalInput")
    xT_d = nc.dram_tensor("xT", [P, R], bf16, kind="ExternalInput")
    ivr_d = nc.dram_tensor("ivr", [P, R], bf16, kind="ExternalInput")
    idx1_d = nc.dram_tensor("idx1", [P, w1], i16, kind="ExternalInput")
    idx2_d = nc.dram_tensor("idx2", [P, w2], i16, kind="ExternalInput")
    dst1_d = nc.dram_tensor("dst1", [P, ncol1], bf16, kind="ExternalInput")
    dst2_d = nc.dram_tensor("dst2", [P, ncol2], bf16, kind="ExternalInput")
    iota_d = nc.dram_tensor("iota", [P, P], bf16, kind="ExternalInput")
    w1n_d = nc.dram_tensor("w1n", [IN_F, HID_F], bf16, kind="ExternalInput")
    w1s_d = nc.dram_tensor("w1s", [IN_F, HID_F], bf16, kind="ExternalInput")
    w2n_d = nc.dram_tensor("w2n", [HID_F, OUT_F], bf16, kind="ExternalInput")
    w2s_d = nc.dram_tensor("w2s", [HID_F, OUT_F], bf16, kind="ExternalInput")
    out_d = nc.dram_tensor("out", [OUT_F, R], f32, kind="ExternalOutput")

    cc_in = nc.dram_tensor("cc_in", [R, P], bf16)
    cc_c0 = nc.dram_tensor("cc_c0", [CORES * C0R, P], bf16)
    cc_c1 = nc.dram_tensor("cc_c1", [CORES * C1R, P], bf16)
    groups_all = [list(range(CORES))]

    with tile.TileContext(nc) as tc:
        with (
            tc.tile_pool(name="pers", bufs=1) as pers,
            tc.tile_pool(name="glo", bufs=3) as glo_pool,
            tc.tile_pool(name="ghi", bufs=3) as ghi_pool,
            tc.tile_pool(name="sone", bufs=4) as spool,
            tc.tile_pool(name="stage", bufs=3) as stage,
            tc.tile_pool(name="pagg", bufs=2, space="PSUM") as pagg_pool,
            tc.tile_pool(name="pproj", bufs=2, space="PSUM") as pproj_pool,
            tc.tile_pool(name="pp2", bufs=2, space="PSUM") as pp2_pool,
        ):
            def load(name, shape, dt, dram):
                t = pers.tile(shape, dt, tag=name)
                nc.sync.dma_start(out=t[:], in_=dram[:, :])
                return t

            idx1 = pers.tile([P, w1], i16, tag="idx1")
            w1_head = off1[(1, 0)] if NGRP > 1 else w1
            nc.sync.dma_start(out=idx1[:, :w1_head], in_=idx1_d[:, :w1_head])
            dst1 = load("dst1", [P, ncol1], bf16, dst1_d)
            nc.sync.dma_start(out=idx1[:, w1_head:], in_=idx1_d[:, w1_head:])
            iota = load("iota", [P, P], bf16, iota_d)
            idx2 = load("idx2", [P, w2], i16, idx2_d)
            xT = load("xT", [P, R], bf16, xT_d)
            ivr = load("ivr", [P, R], bf16, ivr_d)
            dst2 = load("dst2", [P, ncol2], bf16, dst2_d)
            w1n = load("w1n", [IN_F, HID_F], bf16, w1n_d)
            w1s = load("w1s", [IN_F, HID_F], bf16, w1s_d)
            w2n = load("w2n", [HID_F, OUT_F], bf16, w2n_d)
            w2s = load("w2s", [HID_F, OUT_F], bf16, w2s_d)
            h1T = pers.tile([HID_F, R], bf16)

            def gather_seq(gs_tiles, offs, idxs, tables, order, gtiles,
                           nsplit=1):
                """Emit gathers on the gpsimd queue in `order` into gtiles.
                nsplit > 1 emits each gather as several sub-gathers over
                consecutive tile ranges (finer completion granularity)."""
                for g, s in order:
                    ntile = gs_tiles[(g, s)]
                    if ntile == 0:
                        continue
                    pool = glo_pool if s == 0 else ghi_pool
                    gt = pool.tile([P, ntile * P], bf16, tag=f"g{s}")
                    ns = min(nsplit, ntile)
                    bnds = [ntile * i // ns for i in range(ns + 1)]
                    for a, bnd in zip(bnds[:-1], bnds[1:]):
                        nidx = (bnd - a) * P
                        nc.gpsimd.dma_gather(
                            out_ap=gt[:, a * P : bnd * P].rearrange(
                                "p (t e) -> p t e", e=P),
                            in_ap=tables[s][:, :],
                            idxs_ap=idxs[:, offs[(g, s)] + a * P // 16 :
                                         offs[(g, s)] + bnd * P // 16],
                            num_idxs=nidx, num_idxs_reg=nidx, elem_size=P,
                            single_packet=False)
                    gtiles[(g, s)] = gt

            def make_layer_state(dstT, ncol):
                return {"dstT": dstT, "ncol": ncol, "schunks": {}}

            def run_blocks(state, jobs, gtiles, emit, b0, b1):
                dstT, ncol = state["dstT"], state["ncol"]
                schunks = state["schunks"]

                def s_for(c):
                    ci = c // SCHUNK
                    if ci not in schunks:
                        c0 = ci * SCHUNK
                        kk = min(SCHUNK, ncol - c0)
                        st = spool.tile([P, SCHUNK * P], bf16, tag="s")
                        nc.vector.tensor_tensor(
                            out=st[:, : kk * P].rearrange(
                                "p (t q) -> p t q", t=kk),
                            in0=dstT[:, c0 : c0 + kk].unsqueeze(2)
                                .to_broadcast([P, kk, P]),
                            in1=iota[:].unsqueeze(1).to_broadcast([P, kk, P]),
                            op=eq)
                        schunks[ci] = st
                    return schunks[ci], c % SCHUNK

                for b in range(b0, b1):
                    jl = jobs[b]
                    pg = pagg_pool.tile([P, P], f32, tag="agg")
                    nj = len(jl)
                    for i, (g_, s_, t_, col) in enumerate(jl):
                        gt = gtiles[(g_, s_)]
                        st, cc = s_for(col)
                        nc.tensor.matmul(
                            out=pg[:],
                            lhsT=gt[:, t_ * P : (t_ + 1) * P],
                            rhs=st[:, cc * P : (cc + 1) * P],
                            start=(i == 0), stop=(i == nj - 1))
                    emit(b, pg)

            # ---------------- layer 1 ----------------
            def ag0():
                nc.gpsimd.collective_compute(
                    "AllGather", mybir.AluOpType.bypass,
                    replica_groups=groups_all,
                    ins=[cc_in[0:C0R, :].opt()],
                    outs=[cc_c0.ap().opt()])

            def ag1():
                nc.gpsimd.collective_compute(
                    "AllGather", mybir.AluOpType.bypass,
                    replica_groups=groups_all,
                    ins=[cc_in[C0R:R, :].opt()],
                    outs=[cc_c1.ap().opt()])

            g1 = {}
            g2 = {}
            # boundary group: AG0's chunk covers blocks 0..C0B-1

            def emit1(b, pg):
                ts = stage.tile([P, P], bf16, tag="aggs")
                nc.vector.tensor_tensor(
                    out=ts[:], in0=pg[:],
                    in1=ivr[:, b * P : (b + 1) * P], op=mul)
                hp = pproj_pool.tile([HID_F, P], f32, tag="h1p")
                nc.tensor.matmul(out=hp[:], lhsT=w1n[:], rhs=ts[:],
                                 start=True, stop=False)
                nc.tensor.matmul(out=hp[:], lhsT=w1s[:],
                                 rhs=xT[:, b * P : (b + 1) * P],
                                 start=False, stop=True)
                nc.scalar.activation(out=h1T[:, b * P : (b + 1) * P],
                                     in_=hp[:], func=relu)
                p2 = pp2_pool.tile([P, HID_F], f32, tag="p2")
                nc.tensor.matmul(out=p2[:],
                                 lhsT=h1T[:, b * P : (b + 1) * P],
                                 rhs=w2n[:], start=True, stop=True)
                row = stage.tile([P, P], bf16, tag="row")
                nc.vector.memset(row[:, HID_F:], 0)
                nc.vector.tensor_copy(out=row[:, :HID_F], in_=p2[:])
                nc.sync.dma_start(out=cc_in[b * P : (b + 1) * P, :],
                                  in_=row[:])

            st1 = make_layer_state(dst1, ncol1)
            st2 = make_layer_state(dst2, ncol2)

            gather_seq(gt1, off1, idx1, (xlo_d, xhi_d),
                       [(g, s) for g in range(NGRP) for s in range(2)], g1)
            run_blocks(st1, jobs1, g1, emit1, 0, C0B)
            # AG0 fires once blocks 0..C0B-1 rows are written
            ag0()
            early = [(g, 0) for g in range(min(3, NGRP))]
            gather_seq(gt2, off2, idx2, (cc_c0, cc_c1), early, g2)
            run_blocks(st1, jobs1, g1, emit1, C0B, BPC)
            ag1()
            rest0 = [(g, 0) for g in range(3, NGRP)]
            rest1 = [(g, 1) for g in range(NGRP)]
            rest = []
            for i in range(max(len(rest0), len(rest1))):
                if i < len(rest0):
                    rest.append(rest0[i])
                if i < len(rest1):
                    rest.append(rest1[i])
            gather_seq(gt2, off2, idx2, (cc_c0, cc_c1), rest[:-2], g2)
            # trailing gathers split finer so the last blocks' data lands
            # incrementally (shorter tail)
            gather_seq(gt2, off2, idx2, (cc_c0, cc_c1), rest[-2:], g2,
                       nsplit=3)

            # ---------------- layer 2 ----------------

            def emit2(b, pg):
                o2 = pproj_pool.tile([HID_F, P], f32, tag="o2")
                nc.tensor.matmul(out=o2[:], lhsT=w2s[:],
                                 rhs=h1T[:, b * P : (b + 1) * P],
                                 start=True, stop=True)
                nT = stage.tile([HID_F, P], f32, tag="nT")
                nc.vector.tensor_tensor(
                    out=nT[:], in0=pg[:HID_F, :],
                    in1=ivr[:HID_F, b * P : (b + 1) * P], op=mul)
                of = stage.tile([HID_F, P], f32, tag="of")
                nc.vector.tensor_add(out=of[:], in0=nT[:], in1=o2[:])
                oo = stage.tile([HID_F, P], f32, tag="oo")
                nc.scalar.activation(out=oo[:], in_=of[:], func=relu)
                nc.sync.dma_start(out=out_d[:, b * P : (b + 1) * P],
                                  in_=oo[:])

            run_blocks(st2, jobs2, g2, emit2, 0, BPC)

    nc.compile()
    return nc


def _run(inputs, trace=False, tmpdir=None):
    from concourse.bass_utils import run_bass_kernel_spmd

    x = np.asarray(inputs["x"], np.float32)
    src = np.asarray(inputs["src"])
    dst = np.asarray(inputs["dst"])
    b1 = np.asarray(inputs["b1"], np.float32)
    b2 = np.asarray(inputs["b2"], np.float32)
    assert not np.any(b1) and not np.any(b2), "nonzero bias unsupported"

    pos, L1, L2, xtab_lo, xtab_hi, percore = _prep(x, src, dst)

    def hkey(L):
        Tgs, rng = L
        return (tuple(sorted(Tgs.items())),
                tuple(sorted((k, v) for k, v in rng.items())))

    key = (hkey(L1), hkey(L2))
    if key not in _cache:
        _cache[key] = _build(L1, L2)
    nc = _cache[key]

    bf = ml_dtypes.bfloat16
    iota = np.broadcast_to(np.arange(P, dtype=np.float32),
                           (P, P)).astype(bf).copy()
    shared = {
        "xlo": xtab_lo, "xhi": xtab_hi, "iota": iota,
        "w1n": np.asarray(inputs["W1_neigh"], np.float32).astype(bf),
        "w1s": np.asarray(inputs["W1_self"], np.float32).astype(bf),
        "w2n": np.asarray(inputs["W2_neigh"], np.float32).astype(bf),
        "w2s": np.asarray(inputs["W2_self"], np.float32).astype(bf),
    }
    in_maps = []
    for c in range(CORES):
        i1, d1, i2, d2, xT, ivr = percore[c]
        m = dict(shared)
        m.update({"idx1": i1, "dst1": d1, "idx2": i2, "dst2": d2,
                  "xT": xT, "ivr": ivr})
        in_maps.append(m)

    res = run_bass_kernel_spmd(nc, in_maps, list(range(CORES)),
                               trace=trace, tmpdir=tmpdir)
    h2 = np.concatenate([res.results[c]["out"] for c in range(CORES)],
                        axis=1).T  # [NPAD, 64]
    out = h2[pos]
    return np.ascontiguousarray(out, dtype=np.float32), res


def kernel(**inputs) -> np.ndarray:
    out, _ = _run(inputs, trace=False)
    return out
